# revision 18
# baseline (speedup 1.0000x reference)
"""Trainium2 Bass kernel: BiLSTM dependency-parser edge scorer.

Self-contained. Accepts FULL inputs (as produced by setup_inputs()), returns
the FULL [65025, 1] float32 score tensor.

Strategy (per NeuronCore, SPMD over 8 cores; replicated except the edge-score
row selection):
  - The LSTM recurrences are solved by Jacobi fixed-point iteration over the
    time-unrolled network: sweep k computes gates = xg + Whh @ H^(k-1) for ALL
    256 timesteps as batched matmuls (h-feedback lagged one sweep), applies
    sigmoid/tanh as wide activation ops, runs the c-recurrence
    c_t = sigmoid(f_t) * c_{t-1} + u_t with the DVE tensor_tensor_scan
    instruction (a native per-partition linear recurrence along the free dim;
    the backward direction uses reversed access-pattern views), and rebuilds
    h = sigmoid(o) * tanh(c) in one vector op. Each sweep makes h_t exact for
    t < k and contracts the remaining error ~2x; K sweeps per layer suffice
    for the 2e-2 tolerance.
  - Gate layout: 16 tiles of 100 rows, tile = 4*gate_group + j with gate-group
    order (i, g, f, o) so each activation op covers a contiguous column range
    and sigmoid(f) lands before sigmoid(o) on the critical path. Weights are pre-tiled on the host into [100, .] stationary
    operands.
  - H is stored transposed ([100 hidden, 4 j-blocks, 258] with zero guard
    columns) so the same tile serves as the shifted matmul rhs for both
    directions and as rhs chunks for the next layer's input projection and
    the edge-scorer GEMMs.
  - Edge MLP: scores[h,m] = w2 . tanh(A[h] + B[m] + b1) + b2 with
    A = h1 @ Uh^T, B = h1 @ Um^T. Each core computes a [32, 256] slice of the
    score grid (rows picked by a per-core one-hot input); host assembles.
"""

import os
import sys

sys.path.insert(0, "/opt/trn_rl_repo")

import numpy as np

import concourse.bass as bass
import concourse.mybir as mybir
from concourse import bacc
from concourse.bass import IndirectOffsetOnAxis
from concourse.masks import make_identity
from concourse.tile import TileContext

N = 256          # sequence length
NC = 8           # cores
F32 = mybir.dt.float32
BF16 = mybir.dt.float16
I32 = mybir.dt.int32
AF = mybir.ActivationFunctionType
OP = mybir.AluOpType

K_SWEEPS = int(os.environ.get("DP_K", "7"))

# tile-group order (i, g, f, o): sg cols i 0:1024, tanh(g) 1024:2048,
# sigmoid(f) 2048:3072, sigmoid(o) 3072:4096
_OG = (0, 2, 1, 3)


# ---------------------------------------------------------------------------
# host-side weight layout prep
# ---------------------------------------------------------------------------


def _bf(a):
    return np.ascontiguousarray(np.asarray(a).astype(np.float16))


def _rows(tt):
    """Original gate-row indices (torch order i,f,g,o) for tile tt."""
    return 400 * _OG[tt // 4] + 100 * (tt % 4) + np.arange(100)


def _whh_lay(W):
    """W [1600, 400] -> [100 k, 6400] with free = 400*tt + 100*j + m."""
    out = np.zeros((100, 6400), np.float64)
    for tt in range(16):
        R = np.asarray(W, np.float64)[_rows(tt)]      # [100 m, 400]
        for j in range(4):
            out[:, 400 * tt + 100 * j: 400 * tt + 100 * j + 100] = \
                R[:, 100 * j: 100 * j + 100].T
    return out


def _wih_lay(W, nch):
    """W [1600, 100*nch] -> [100 k, 1600*nch/16*...]: free = (100*nch)*tt + 100*ch + m."""
    D = 100 * nch
    out = np.zeros((100, 16 * D), np.float64)
    for tt in range(16):
        R = np.asarray(W, np.float64)[_rows(tt)]      # [100 m, D]
        for ch in range(nch):
            out[:, D * tt + 100 * ch: D * tt + 100 * ch + 100] = \
                R[:, 100 * ch: 100 * ch + 100].T
    return out


def _bias_lay(b):
    """b [1600] -> [1600] with index 100*tt + m."""
    out = np.zeros(1600, np.float64)
    for tt in range(16):
        out[100 * tt: 100 * tt + 100] = np.asarray(b, np.float64)[_rows(tt)]
    return out


def _prep_inputs(word_idx, pos_idx, word_emb, pos_emb,
                 Wih0, Whh0, bih0, bhh0, Wih1, Whh1, bih1, bhh1,
                 fc1_W, fc1_b, fc2_W, fc2_b):
    arr = {}
    arr["widx"] = np.ascontiguousarray(
        np.asarray(word_idx).reshape(N, 1).astype(np.int32))
    arr["pidx"] = np.ascontiguousarray(
        np.asarray(pos_idx).reshape(N, 1).astype(np.int32))
    arr["wemb"] = np.ascontiguousarray(np.asarray(word_emb, dtype=np.float32))
    arr["pemb"] = np.ascontiguousarray(np.asarray(pos_emb, dtype=np.float32))

    whh = np.zeros((4, 100, 6400), np.float64)
    wih0 = np.zeros((2, 100, 6400), np.float64)
    wih1 = np.zeros((2, 100, 12800), np.float64)
    bias = np.zeros((2, 3200), np.float64)
    for d in range(2):
        whh[2 * 0 + d] = _whh_lay(np.asarray(Whh0)[d])
        whh[2 * 1 + d] = _whh_lay(np.asarray(Whh1)[d])
        wih0[d] = _wih_lay(np.asarray(Wih0)[d], 4)
        wih1[d] = _wih_lay(np.asarray(Wih1)[d], 8)
        bias[0, 1600 * d: 1600 * d + 1600] = _bias_lay(
            np.asarray(bih0)[d] + np.asarray(bhh0)[d])
        bias[1, 1600 * d: 1600 * d + 1600] = _bias_lay(
            np.asarray(bih1)[d] + np.asarray(bhh1)[d])
    arr["whh"] = _bf(whh)
    arr["wih0"] = _bf(wih0)
    arr["wih1"] = _bf(wih1)
    arr["bias0"] = _bf(bias[0:1])
    arr["bias1"] = _bf(bias[1:2])
    arr["idn100"] = _bf(np.eye(100))

    # edge MLP: Uh = fc1_W[:, :800].T chunks, Um = fc1_W[:, 800:].T chunks
    f1 = np.asarray(fc1_W, np.float64)
    uh = np.zeros((100, 800), np.float64)
    um = np.zeros((100, 800), np.float64)
    for c in range(8):
        uh[:, 100 * c: 100 * c + 100] = f1[:, 100 * c: 100 * c + 100].T
        um[:, 100 * c: 100 * c + 100] = f1[:, 800 + 100 * c: 900 + 100 * c].T
    arr["uh"] = _bf(uh)
    arr["um"] = _bf(um)
    arr["w2"] = _bf(np.asarray(fc2_W, np.float64).reshape(100, 1))
    arr["b1"] = np.ascontiguousarray(
        np.asarray(fc1_b, np.float32).reshape(100, 1))
    arr["b2"] = np.ascontiguousarray(
        np.full((128, 1), np.float32(np.asarray(fc2_b).reshape(())),
                dtype=np.float32))
    return arr


def _make_selT(core):
    s = np.zeros((2, 128, 32), np.float32)
    for r in range(32):
        t = 32 * core + r
        s[t // 128, t % 128, r] = 1.0
    return np.ascontiguousarray(s)


# ---------------------------------------------------------------------------
# device kernel build
# ---------------------------------------------------------------------------


def build_nc():
    nc = bacc.Bacc("TRN2", target_bir_lowering=False, debug=False,
                   num_devices=NC)
    wemb = nc.dram_tensor("wemb", [50000, 300], F32, kind="ExternalInput").ap()
    pemb = nc.dram_tensor("pemb", [50, 100], F32, kind="ExternalInput").ap()
    widx = nc.dram_tensor("widx", [N, 1], I32, kind="ExternalInput").ap()
    pidx = nc.dram_tensor("pidx", [N, 1], I32, kind="ExternalInput").ap()
    whhd = nc.dram_tensor("whh", [4, 100, 6400], BF16, kind="ExternalInput").ap()
    wih0d = nc.dram_tensor("wih0", [2, 100, 6400], BF16, kind="ExternalInput").ap()
    wih1d = nc.dram_tensor("wih1", [2, 100, 12800], BF16, kind="ExternalInput").ap()
    bias0d = nc.dram_tensor("bias0", [1, 3200], BF16, kind="ExternalInput").ap()
    bias1d = nc.dram_tensor("bias1", [1, 3200], BF16, kind="ExternalInput").ap()
    idnd = nc.dram_tensor("idn100", [100, 100], BF16, kind="ExternalInput").ap()
    uhd = nc.dram_tensor("uh", [100, 800], BF16, kind="ExternalInput").ap()
    umd = nc.dram_tensor("um", [100, 800], BF16, kind="ExternalInput").ap()
    w2d = nc.dram_tensor("w2", [100, 1], BF16, kind="ExternalInput").ap()
    b1d = nc.dram_tensor("b1", [100, 1], F32, kind="ExternalInput").ap()
    b2d = nc.dram_tensor("b2", [128, 1], F32, kind="ExternalInput").ap()
    selTd = nc.dram_tensor("selT", [2, 128, 32], F32, kind="ExternalInput").ap()
    grid = nc.dram_tensor("grid", [32, N], F32, kind="ExternalOutput").ap()

    from contextlib import ExitStack
    with TileContext(nc) as tc, ExitStack() as ctx:
        top = ctx.enter_context(tc.tile_pool(name="top", bufs=1))
        # persistent weights
        whh_sb = [top.tile([100, 6400], BF16, name=f"whh{dl}", tag=f"whh{dl}")
                  for dl in range(4)]
        wih1_sb = [top.tile([100, 12800], BF16, name=f"wih1_{d}", tag=f"wih1_{d}")
                   for d in range(2)]
        bias_sb = [top.tile([1, 3200], BF16, name=f"bias{l}", tag=f"bias{l}")
                   for l in range(2)]
        idn100 = top.tile([100, 100], BF16, name="idn100", tag="idn100")
        idn128 = top.tile([128, 128], F32, name="idn128", tag="idn128")
        make_identity(nc, idn128[:, :])
        ones_sb = top.tile([1, N], BF16, name="ones", tag="ones")
        nc.gpsimd.memset(ones_sb[:, :], 1.0)
        # xg (input projections + bias), tile-major cols: 256*tt + t
        xgT = [[top.tile([100, 4096], BF16, name=f"xg{l}{d}", tag=f"xg{l}{d}")
                for d in range(2)] for l in range(2)]
        # H state, [100, 4 j, 258] with guard cols 0 and 257
        H = [[top.tile([100, 4, 258], BF16, name=f"H{l}{d}", tag=f"H{l}{d}")
              for d in range(2)] for l in range(2)]
        for l in range(2):
            for d in range(2):
                nc.gpsimd.memset(H[l][d][:, :, :], 0.0)
        # edge weights
        uh_sb = top.tile([100, 800], BF16, name="uh", tag="uh")
        um_sb = top.tile([100, 800], BF16, name="um", tag="um")
        w2_sb = top.tile([100, 1], BF16, name="w2", tag="w2")
        b1_sb = top.tile([100, 1], F32, name="b1", tag="b1")
        b2_sb = top.tile([128, 1], F32, name="b2", tag="b2")
        selT_sb = top.tile([128, 64], F32, name="selT", tag="selT")
        xT = top.tile([100, 1024], BF16, name="xT", tag="xT")

        # =========== embedding gather + transpose -> xT ===========
        # DMA queue priority: idx first (unblocks the gathers), then wih0
        # (first GEMM), then the small weights, then whh; wih1 rides the ACT
        # engine's DMA queue in parallel.
        w0ctx = tc.tile_pool(name="wih0p", bufs=1)
        w0p = w0ctx.__enter__()
        wih0_sb = [w0p.tile([100, 6400], BF16, name=f"wih0_{d}", tag=f"wih0_{d}")
                   for d in range(2)]
        with tc.tile_pool(name="embed", bufs=1) as epool, \
             tc.tile_pool(name="embps", bufs=2, space="PSUM") as eps:
            idx_sb = epool.tile([128, 4], I32, name="idx", tag="idx")
            nc.sync.dma_start(out=idx_sb[0:128, 0:1], in_=widx[0:128, 0:1])
            nc.sync.dma_start(out=idx_sb[0:128, 1:2], in_=widx[128:256, 0:1])
            nc.sync.dma_start(out=idx_sb[0:128, 2:3], in_=pidx[0:128, 0:1])
            nc.sync.dma_start(out=idx_sb[0:128, 3:4], in_=pidx[128:256, 0:1])
            x_sb = epool.tile([128, 800], F32, name="xsb", tag="xsb")
            for tb in range(2):
                nc.gpsimd.indirect_dma_start(
                    out=x_sb[0:128, 400 * tb: 400 * tb + 300],
                    out_offset=None,
                    in_=wemb[:, :],
                    in_offset=IndirectOffsetOnAxis(
                        ap=idx_sb[0:128, tb:tb + 1], axis=0))
                nc.gpsimd.indirect_dma_start(
                    out=x_sb[0:128, 400 * tb + 300: 400 * tb + 400],
                    out_offset=None,
                    in_=pemb[:, :],
                    in_offset=IndirectOffsetOnAxis(
                        ap=idx_sb[0:128, 2 + tb:3 + tb], axis=0))
            nc.sync.dma_start(out=bias_sb[0][:, :], in_=bias0d[0])
            for d in range(2):
                nc.sync.dma_start(out=wih0_sb[d][:, :], in_=wih0d[d])
            nc.sync.dma_start(out=bias_sb[1][:, :], in_=bias1d[0])
            nc.sync.dma_start(out=idn100[:, :], in_=idnd[:, :])
            nc.sync.dma_start(out=uh_sb[:, :], in_=uhd[:, :])
            nc.sync.dma_start(out=um_sb[:, :], in_=umd[:, :])
            nc.sync.dma_start(out=w2_sb[:, :], in_=w2d[:, :])
            nc.sync.dma_start(out=b1_sb[:, :], in_=b1d[:, :])
            nc.sync.dma_start(out=b2_sb[:, :], in_=b2d[:, :])
            nc.sync.dma_start(out=selT_sb[0:128, 0:32], in_=selTd[0])
            nc.sync.dma_start(out=selT_sb[0:128, 32:64], in_=selTd[1])
            for dl in range(4):
                nc.sync.dma_start(out=whh_sb[dl][:, :], in_=whhd[dl])
            for d in range(2):
                nc.sync.dma_start(out=wih1_sb[d][:, :], in_=wih1d[d])
            for tb in range(2):
                for ch in range(4):
                    ptr = eps.tile([128, 128], F32, name="ptr", tag="ptr")
                    nc.tensor.transpose(
                        out=ptr[0:100, 0:128],
                        in_=x_sb[0:128, 400 * tb + 100 * ch: 400 * tb + 100 * ch + 100],
                        identity=idn128[:, :])
                    nc.vector.tensor_copy(
                        out=xT[0:100, 256 * ch + 128 * tb: 256 * ch + 128 * tb + 128],
                        in_=ptr[0:100, 0:128])

        # =========== xg for layer 0 ===========
        with tc.tile_pool(name="xg0ps", bufs=2, space="PSUM") as xps:
            for d in range(2):
                for half in range(2):
                    ps = xps.tile([128, 2048], F32, name="xg0ps", tag="xg0ps")
                    for tl in range(8):
                        tt = 8 * half + tl
                        for ch in range(4):
                            nc.tensor.matmul(
                                ps[0:100, 256 * tl: 256 * tl + 256],
                                lhsT=wih0_sb[d][0:100, 400 * tt + 100 * ch: 400 * tt + 100 * ch + 100],
                                rhs=xT[0:100, 256 * ch: 256 * ch + 256],
                                start=(ch == 0), stop=False,
                                skip_group_check=True)
                        nc.tensor.matmul(
                            ps[0:100, 256 * tl: 256 * tl + 256],
                            lhsT=bias_sb[0][0:1, 1600 * d + 100 * tt: 1600 * d + 100 * tt + 100],
                            rhs=ones_sb[0:1, 0:256],
                            start=False, stop=True, skip_group_check=True)
                    if half == 0:
                        nc.vector.tensor_copy(
                            out=xgT[0][d][0:100, 0:2048],
                            in_=ps[0:100, 0:2048])
                    elif d == 1:
                        nc.vector.tensor_copy(
                            out=xgT[0][d][0:100, 2048:4096],
                            in_=ps[0:100, 0:2048])
                    else:
                        nc.scalar.copy(
                            out=xgT[0][d][0:100, 2048:4096],
                            in_=ps[0:100, 0:2048])
        w0ctx.__exit__(None, None, None)

        # =========== Jacobi sweep emitter ===========
        def emit_sweeps(l):
            with tc.tile_pool(name=f"sg{l}", bufs=1) as sgp, \
                 tc.tile_pool(name=f"scr{l}", bufs=1) as scr, \
                 tc.tile_pool(name=f"gps{l}", bufs=2, space="PSUM") as gps:
                for k in range(K_SWEEPS):
                    for d in range(2):
                        dl = 2 * l + d
                        sg = sgp.tile([100, 4096], F32, name=f"sg{d}", tag="sg")
                        if k == 0:
                            src = [xgT[l][d][0:100, 0:1024],
                                   xgT[l][d][0:100, 1024:2048],
                                   xgT[l][d][0:100, 2048:3072],
                                   xgT[l][d][0:100, 3072:4096]]
                        else:
                            src = []
                            for half in range(2):
                                ps = gps.tile([128, 2048], F32, name="gps", tag="gps")
                                for q in range(4):
                                    nc.tensor.matmul(
                                        ps[0:100, 512 * q: 512 * q + 512],
                                        lhsT=idn100[0:100, 0:100],
                                        rhs=xgT[l][d][0:100, 2048 * half + 512 * q: 2048 * half + 512 * q + 512],
                                        start=True, stop=False,
                                        skip_group_check=True)
                                for tl in range(8):
                                    tt = 8 * half + tl
                                    for j in range(4):
                                        # h_{t-1} (fwd) / h_{t+1} (bwd) via guard cols
                                        o0 = 0 if d == 0 else 2
                                        nc.tensor.matmul(
                                            ps[0:100, 256 * tl: 256 * tl + 256],
                                            lhsT=whh_sb[dl][0:100, 400 * tt + 100 * j: 400 * tt + 100 * j + 100],
                                            rhs=H[l][d][0:100, j, o0: o0 + 256],
                                            start=False, stop=(j == 3),
                                            skip_group_check=True)
                                src.append(ps[0:100, 0:1024])
                                src.append(ps[0:100, 1024:2048])
                        # i: sigmoid, g: tanh, f: sigmoid (before o), o: sigmoid
                        nc.scalar.activation(sg[0:100, 0:1024], src[0], AF.Sigmoid)
                        nc.scalar.activation(sg[0:100, 1024:2048], src[1], AF.Tanh)
                        nc.scalar.activation(sg[0:100, 2048:3072], src[2], AF.Sigmoid)
                        nc.scalar.activation(sg[0:100, 3072:4096], src[3], AF.Sigmoid)
                        u = scr.tile([100, 1024], F32, name=f"u{d}", tag=f"u{d}")
                        c = scr.tile([100, 1024], F32, name=f"c{d}", tag=f"c{d}")
                        thc = scr.tile([100, 1024], F32, name=f"th{d}", tag=f"th{d}")
                        nc.vector.tensor_tensor(
                            out=u[0:100, 0:1024], in0=sg[0:100, 0:1024],
                            in1=sg[0:100, 1024:2048], op=OP.mult)
                        for j in range(4):
                            if d == 0:
                                nc.vector.tensor_tensor_scan(
                                    out=c[0:100, 256 * j: 256 * j + 256],
                                    data0=sg[0:100, 2048 + 256 * j: 2304 + 256 * j],
                                    data1=u[0:100, 256 * j: 256 * j + 256],
                                    initial=0.0, op0=OP.mult, op1=OP.add)
                            else:
                                e1 = 256 * j - 1
                                nc.vector.tensor_tensor_scan(
                                    out=c[0:100, 256 * j + 255: (e1 if e1 >= 0 else None): -1],
                                    data0=sg[0:100, 2303 + 256 * j: 2047 + 256 * j: -1],
                                    data1=u[0:100, 256 * j + 255: (e1 if e1 >= 0 else None): -1],
                                    initial=0.0, op0=OP.mult, op1=OP.add)
                        nc.scalar.activation(thc[0:100, 0:1024], c[0:100, 0:1024], AF.Tanh)
                        nc.vector.tensor_tensor(
                            out=H[l][d][0:100, 0:4, 1:257],
                            in0=sg[0:100, 3072:4096], in1=thc[0:100, 0:1024],
                            op=OP.mult)

        emit_sweeps(0)

        # =========== xg for layer 1 (from H0) ===========
        with tc.tile_pool(name="xg1ps", bufs=2, space="PSUM") as xps:
            for d in range(2):
                for half in range(2):
                    ps = xps.tile([128, 2048], F32, name="xg1ps", tag="xg1ps")
                    for tl in range(8):
                        tt = 8 * half + tl
                        for ch in range(8):
                            dd, j = divmod(ch, 4)
                            nc.tensor.matmul(
                                ps[0:100, 256 * tl: 256 * tl + 256],
                                lhsT=wih1_sb[d][0:100, 800 * tt + 100 * ch: 800 * tt + 100 * ch + 100],
                                rhs=H[0][dd][0:100, j, 1:257],
                                start=(ch == 0), stop=False,
                                skip_group_check=True)
                        nc.tensor.matmul(
                            ps[0:100, 256 * tl: 256 * tl + 256],
                            lhsT=bias_sb[1][0:1, 1600 * d + 100 * tt: 1600 * d + 100 * tt + 100],
                            rhs=ones_sb[0:1, 0:256],
                            start=False, stop=True, skip_group_check=True)
                    if half == 0:
                        nc.vector.tensor_copy(
                            out=xgT[1][d][0:100, 0:2048], in_=ps[0:100, 0:2048])
                    elif d == 1:
                        nc.vector.tensor_copy(
                            out=xgT[1][d][0:100, 2048:4096], in_=ps[0:100, 0:2048])
                    else:
                        nc.scalar.copy(
                            out=xgT[1][d][0:100, 2048:4096], in_=ps[0:100, 0:2048])

        emit_sweeps(1)

        # =========== edge scorer ===========
        with tc.tile_pool(name="edge", bufs=1) as ep, \
             tc.tile_pool(name="edgeth", bufs=3) as thp, \
             tc.tile_pool(name="edgeps", bufs=1, space="PSUM") as epps, \
             tc.tile_pool(name="edgept", bufs=1, space="PSUM") as ptps:
            # B^T [100 f, 256 m] = Um^T @ h1cat (b1 folded into A side)
            pB = epps.tile([128, 256], F32, name="pB", tag="pB")
            for c in range(8):
                dd, j = divmod(c, 4)
                nc.tensor.matmul(
                    pB[0:100, 0:256],
                    lhsT=um_sb[0:100, 100 * c: 100 * c + 100],
                    rhs=H[1][dd][0:100, j, 1:257],
                    start=(c == 0), stop=(c == 7))
            # A^T [100 f, 256 t]
            pA = epps.tile([128, 256], F32, name="pA", tag="pA")
            for c in range(8):
                dd, j = divmod(c, 4)
                nc.tensor.matmul(
                    pA[0:100, 0:256],
                    lhsT=uh_sb[0:100, 100 * c: 100 * c + 100],
                    rhs=H[1][dd][0:100, j, 1:257],
                    start=(c == 0), stop=(c == 7))
            A_sb = ep.tile([100, 256], F32, name="A", tag="A")
            nc.vector.tensor_copy(out=A_sb[0:100, 0:256], in_=pA[0:100, 0:256])
            # select this core's 32 rows: transpose A^T chunks then selT matmul
            At_sb = ep.tile([128, 256], F32, name="At", tag="At")
            for m in range(2):
                pt = ptps.tile([128, 128], F32, name="pt", tag="pt")
                nc.tensor.transpose(
                    out=pt[0:128, 0:100],
                    in_=A_sb[0:100, 128 * m: 128 * m + 128],
                    identity=idn128[0:100, 0:100])
                nc.vector.tensor_copy(
                    out=At_sb[0:128, 128 * m: 128 * m + 100],
                    in_=pt[0:128, 0:100])
            pS = ptps.tile([128, 32], F32, name="pS", tag="pS")
            for m in range(2):
                nc.tensor.matmul(
                    pS[0:100, 0:32],
                    lhsT=At_sb[0:128, 128 * m: 128 * m + 100],
                    rhs=selT_sb[0:128, 32 * m: 32 * m + 32],
                    start=(m == 0), stop=(m == 1))
            ATb = ep.tile([100, 32], F32, name="ATb", tag="ATb")
            nc.vector.tensor_scalar(
                out=ATb[0:100, 0:32], in0=pS[0:100, 0:32],
                scalar1=b1_sb[0:100, 0:1], scalar2=None, op0=OP.add)

            psS_tiles = [epps.tile([128, 512], F32, name=f"psS{q}", tag=f"psS{q}")
                         for q in range(4)]
            for q in range(4):
                nc.vector.memset(psS_tiles[q][:, :], 0.0)
            gsb_tiles = [ep.tile([128, 512], F32, name=f"gsb{q}", tag=f"gsb{q}")
                         for q in range(4)]
            for r in range(32):
                th_t = thp.tile([100, 256], BF16, name="th", tag="th")
                nc.scalar.activation(
                    th_t[0:100, 0:256], pB[0:100, 0:256], AF.Tanh,
                    bias=ATb[0:100, r:r + 1], scale=1.0)
                q, half = divmod(r // 4, 2)
                nc.tensor.matmul(
                    psS_tiles[q][32 * (r % 4): 32 * (r % 4) + 1,
                                 256 * half: 256 * half + 256],
                    lhsT=w2_sb[0:100, 0:1],
                    rhs=th_t[0:100, 0:256],
                    start=True, stop=True,
                    skip_group_check=True,
                    tile_position=(0, 32 * (r % 4)))
                if r % 8 == 7:
                    # quadrant q complete -> write back while later rows run
                    nc.vector.tensor_scalar(
                        out=gsb_tiles[q][0:128, 0:512],
                        in0=psS_tiles[q][0:128, 0:512],
                        scalar1=b2_sb[0:128, 0:1], scalar2=None, op0=OP.add)
                    for hh in range(2):
                        rb = 4 * (2 * q + hh)
                        nc.sync.dma_start(
                            out=grid[rb:rb + 4, 0:256],
                            in_=gsb_tiles[q][0:128:32, 256 * hh: 256 * hh + 256])

    nc.compile()
    return nc


_NC_CACHE = None


def _get_nc():
    global _NC_CACHE
    if _NC_CACHE is None:
        _NC_CACHE = build_nc()
    return _NC_CACHE


def kernel(**inputs) -> np.ndarray:
    from concourse.bass_utils import run_bass_kernel_spmd

    arr = _prep_inputs(**inputs)
    nc = _get_nc()
    in_maps = []
    for k in range(NC):
        m = dict(arr)
        m["selT"] = _make_selT(k)
        in_maps.append(m)
    res = run_bass_kernel_spmd(nc, in_maps, core_ids=list(range(NC)))
    grid = np.concatenate([res.results[k]["grid"] for k in range(NC)], axis=0)
    mask = np.ones((N, N), dtype=bool)
    np.fill_diagonal(mask, False)
    mask[:, 0] = False
    return grid[mask].reshape(-1, 1).astype(np.float32)


# revision 19
# speedup vs baseline: 1.0028x; 1.0028x over previous
"""Trainium2 Bass kernel: BiLSTM dependency-parser edge scorer.

Self-contained. Accepts FULL inputs (as produced by setup_inputs()), returns
the FULL [65025, 1] float32 score tensor.

Strategy (per NeuronCore, SPMD over 8 cores; replicated except the edge-score
row selection):
  - The LSTM recurrences are solved by Jacobi fixed-point iteration over the
    time-unrolled network: sweep k computes gates = xg + Whh @ H^(k-1) for ALL
    256 timesteps as batched matmuls (h-feedback lagged one sweep), applies
    sigmoid/tanh as wide activation ops, runs the c-recurrence
    c_t = sigmoid(f_t) * c_{t-1} + u_t with the DVE tensor_tensor_scan
    instruction (a native per-partition linear recurrence along the free dim;
    the backward direction uses reversed access-pattern views), and rebuilds
    h = sigmoid(o) * tanh(c) in one vector op. Each sweep makes h_t exact for
    t < k and contracts the remaining error ~2x; K sweeps per layer suffice
    for the 2e-2 tolerance.
  - Gate layout: 16 tiles of 100 rows, tile = 4*gate_group + j with gate-group
    order (i, g, f, o) so each activation op covers a contiguous column range
    and sigmoid(f) lands before sigmoid(o) on the critical path. Weights are pre-tiled on the host into [100, .] stationary
    operands.
  - H is stored transposed ([100 hidden, 4 j-blocks, 258] with zero guard
    columns) so the same tile serves as the shifted matmul rhs for both
    directions and as rhs chunks for the next layer's input projection and
    the edge-scorer GEMMs.
  - Edge MLP: scores[h,m] = w2 . tanh(A[h] + B[m] + b1) + b2 with
    A = h1 @ Uh^T, B = h1 @ Um^T. Each core computes a [32, 256] slice of the
    score grid (rows picked by a per-core one-hot input); host assembles.
"""

import os
import sys

sys.path.insert(0, "/opt/trn_rl_repo")

import numpy as np

import concourse.bass as bass
import concourse.mybir as mybir
from concourse import bacc
from concourse.bass import IndirectOffsetOnAxis
from concourse.masks import make_identity
from concourse.tile import TileContext

N = 256          # sequence length
NC = 8           # cores
F32 = mybir.dt.float32
BF16 = mybir.dt.float16
I32 = mybir.dt.int32
AF = mybir.ActivationFunctionType
OP = mybir.AluOpType

K_SWEEPS = int(os.environ.get("DP_K", "7"))

# tile-group order (i, g, f, o): sg cols i 0:1024, tanh(g) 1024:2048,
# sigmoid(f) 2048:3072, sigmoid(o) 3072:4096
_OG = (0, 2, 1, 3)


# ---------------------------------------------------------------------------
# host-side weight layout prep
# ---------------------------------------------------------------------------


def _bf(a):
    return np.ascontiguousarray(np.asarray(a).astype(np.float16))


def _rows(tt):
    """Original gate-row indices (torch order i,f,g,o) for tile tt."""
    return 400 * _OG[tt // 4] + 100 * (tt % 4) + np.arange(100)


def _whh_lay(W):
    """W [1600, 400] -> [100 k, 6400] with free = 400*tt + 100*j + m."""
    out = np.zeros((100, 6400), np.float64)
    for tt in range(16):
        R = np.asarray(W, np.float64)[_rows(tt)]      # [100 m, 400]
        for j in range(4):
            out[:, 400 * tt + 100 * j: 400 * tt + 100 * j + 100] = \
                R[:, 100 * j: 100 * j + 100].T
    return out


def _wih_lay(W, nch):
    """W [1600, 100*nch] -> [100 k, 1600*nch/16*...]: free = (100*nch)*tt + 100*ch + m."""
    D = 100 * nch
    out = np.zeros((100, 16 * D), np.float64)
    for tt in range(16):
        R = np.asarray(W, np.float64)[_rows(tt)]      # [100 m, D]
        for ch in range(nch):
            out[:, D * tt + 100 * ch: D * tt + 100 * ch + 100] = \
                R[:, 100 * ch: 100 * ch + 100].T
    return out


def _bias_lay(b):
    """b [1600] -> [1600] with index 100*tt + m."""
    out = np.zeros(1600, np.float64)
    for tt in range(16):
        out[100 * tt: 100 * tt + 100] = np.asarray(b, np.float64)[_rows(tt)]
    return out


def _prep_inputs(word_idx, pos_idx, word_emb, pos_emb,
                 Wih0, Whh0, bih0, bhh0, Wih1, Whh1, bih1, bhh1,
                 fc1_W, fc1_b, fc2_W, fc2_b):
    arr = {}
    arr["widx"] = np.ascontiguousarray(
        np.asarray(word_idx).reshape(N, 1).astype(np.int32))
    arr["pidx"] = np.ascontiguousarray(
        np.asarray(pos_idx).reshape(N, 1).astype(np.int32))
    arr["wemb"] = np.ascontiguousarray(np.asarray(word_emb, dtype=np.float32))
    arr["pemb"] = np.ascontiguousarray(np.asarray(pos_emb, dtype=np.float32))

    whh = np.zeros((4, 100, 6400), np.float64)
    wih0 = np.zeros((2, 100, 6400), np.float64)
    wih1 = np.zeros((2, 100, 12800), np.float64)
    bias = np.zeros((2, 3200), np.float64)
    for d in range(2):
        whh[2 * 0 + d] = _whh_lay(np.asarray(Whh0)[d])
        whh[2 * 1 + d] = _whh_lay(np.asarray(Whh1)[d])
        wih0[d] = _wih_lay(np.asarray(Wih0)[d], 4)
        wih1[d] = _wih_lay(np.asarray(Wih1)[d], 8)
        bias[0, 1600 * d: 1600 * d + 1600] = _bias_lay(
            np.asarray(bih0)[d] + np.asarray(bhh0)[d])
        bias[1, 1600 * d: 1600 * d + 1600] = _bias_lay(
            np.asarray(bih1)[d] + np.asarray(bhh1)[d])
    arr["whh"] = _bf(whh)
    arr["wih0"] = _bf(wih0)
    arr["wih1"] = _bf(wih1)
    arr["bias0"] = _bf(bias[0:1])
    arr["bias1"] = _bf(bias[1:2])
    arr["idn100"] = _bf(np.eye(100))

    # edge MLP: Uh = fc1_W[:, :800].T chunks, Um = fc1_W[:, 800:].T chunks
    f1 = np.asarray(fc1_W, np.float64)
    uh = np.zeros((100, 800), np.float64)
    um = np.zeros((100, 800), np.float64)
    for c in range(8):
        uh[:, 100 * c: 100 * c + 100] = f1[:, 100 * c: 100 * c + 100].T
        um[:, 100 * c: 100 * c + 100] = f1[:, 800 + 100 * c: 900 + 100 * c].T
    arr["uh"] = _bf(uh)
    arr["um"] = _bf(um)
    arr["w2"] = _bf(np.asarray(fc2_W, np.float64).reshape(100, 1))
    arr["b1"] = np.ascontiguousarray(
        np.asarray(fc1_b, np.float32).reshape(100, 1))
    arr["b2"] = np.ascontiguousarray(
        np.full((128, 1), np.float32(np.asarray(fc2_b).reshape(())),
                dtype=np.float32))
    return arr


def _make_selT(core):
    s = np.zeros((2, 128, 32), np.float32)
    for r in range(32):
        t = 32 * core + r
        s[t // 128, t % 128, r] = 1.0
    return np.ascontiguousarray(s)


# ---------------------------------------------------------------------------
# device kernel build
# ---------------------------------------------------------------------------


def build_nc():
    nc = bacc.Bacc("TRN2", target_bir_lowering=False, debug=False,
                   num_devices=NC)
    wemb = nc.dram_tensor("wemb", [50000, 300], F32, kind="ExternalInput").ap()
    pemb = nc.dram_tensor("pemb", [50, 100], F32, kind="ExternalInput").ap()
    widx = nc.dram_tensor("widx", [N, 1], I32, kind="ExternalInput").ap()
    pidx = nc.dram_tensor("pidx", [N, 1], I32, kind="ExternalInput").ap()
    whhd = nc.dram_tensor("whh", [4, 100, 6400], BF16, kind="ExternalInput").ap()
    wih0d = nc.dram_tensor("wih0", [2, 100, 6400], BF16, kind="ExternalInput").ap()
    wih1d = nc.dram_tensor("wih1", [2, 100, 12800], BF16, kind="ExternalInput").ap()
    bias0d = nc.dram_tensor("bias0", [1, 3200], BF16, kind="ExternalInput").ap()
    bias1d = nc.dram_tensor("bias1", [1, 3200], BF16, kind="ExternalInput").ap()
    idnd = nc.dram_tensor("idn100", [100, 100], BF16, kind="ExternalInput").ap()
    uhd = nc.dram_tensor("uh", [100, 800], BF16, kind="ExternalInput").ap()
    umd = nc.dram_tensor("um", [100, 800], BF16, kind="ExternalInput").ap()
    w2d = nc.dram_tensor("w2", [100, 1], BF16, kind="ExternalInput").ap()
    b1d = nc.dram_tensor("b1", [100, 1], F32, kind="ExternalInput").ap()
    b2d = nc.dram_tensor("b2", [128, 1], F32, kind="ExternalInput").ap()
    selTd = nc.dram_tensor("selT", [2, 128, 32], F32, kind="ExternalInput").ap()
    grid = nc.dram_tensor("grid", [32, N], F32, kind="ExternalOutput").ap()

    from contextlib import ExitStack
    with TileContext(nc) as tc, ExitStack() as ctx:
        top = ctx.enter_context(tc.tile_pool(name="top", bufs=1))
        # persistent weights
        whh_sb = [top.tile([100, 6400], BF16, name=f"whh{dl}", tag=f"whh{dl}")
                  for dl in range(4)]
        wih1_sb = [top.tile([100, 12800], BF16, name=f"wih1_{d}", tag=f"wih1_{d}")
                   for d in range(2)]
        bias_sb = [top.tile([1, 3200], BF16, name=f"bias{l}", tag=f"bias{l}")
                   for l in range(2)]
        idn100 = top.tile([100, 100], BF16, name="idn100", tag="idn100")
        idn128 = top.tile([128, 128], F32, name="idn128", tag="idn128")
        make_identity(nc, idn128[:, :])
        ones_sb = top.tile([1, N], BF16, name="ones", tag="ones")
        nc.gpsimd.memset(ones_sb[:, :], 1.0)
        # xg (input projections + bias), tile-major cols: 256*tt + t
        xgT = [[top.tile([100, 4096], BF16, name=f"xg{l}{d}", tag=f"xg{l}{d}")
                for d in range(2)] for l in range(2)]
        # H state, [100, 4 j, 258] with guard cols 0 and 257
        H = [[top.tile([100, 4, 258], BF16, name=f"H{l}{d}", tag=f"H{l}{d}")
              for d in range(2)] for l in range(2)]
        for l in range(2):
            for d in range(2):
                nc.gpsimd.memset(H[l][d][:, :, :], 0.0)
        # edge weights
        uh_sb = top.tile([100, 800], BF16, name="uh", tag="uh")
        um_sb = top.tile([100, 800], BF16, name="um", tag="um")
        w2_sb = top.tile([100, 1], BF16, name="w2", tag="w2")
        b1_sb = top.tile([100, 1], F32, name="b1", tag="b1")
        b2_sb = top.tile([128, 1], F32, name="b2", tag="b2")
        selT_sb = top.tile([128, 64], F32, name="selT", tag="selT")
        xT = top.tile([100, 1024], BF16, name="xT", tag="xT")

        # =========== embedding gather + transpose -> xT ===========
        # DMA queue priority: idx first (unblocks the gathers), then wih0
        # (first GEMM), then the small weights, then whh; wih1 rides the ACT
        # engine's DMA queue in parallel.
        w0ctx = tc.tile_pool(name="wih0p", bufs=1)
        w0p = w0ctx.__enter__()
        wih0_sb = [w0p.tile([100, 6400], BF16, name=f"wih0_{d}", tag=f"wih0_{d}")
                   for d in range(2)]
        with tc.tile_pool(name="embed", bufs=1) as epool, \
             tc.tile_pool(name="embps", bufs=2, space="PSUM") as eps:
            idx_sb = epool.tile([128, 4], I32, name="idx", tag="idx")
            nc.sync.dma_start(out=idx_sb[0:128, 0:1], in_=widx[0:128, 0:1])
            nc.sync.dma_start(out=idx_sb[0:128, 1:2], in_=widx[128:256, 0:1])
            nc.sync.dma_start(out=idx_sb[0:128, 2:3], in_=pidx[0:128, 0:1])
            nc.sync.dma_start(out=idx_sb[0:128, 3:4], in_=pidx[128:256, 0:1])
            x_sb = epool.tile([128, 800], F32, name="xsb", tag="xsb")
            for tb in range(2):
                nc.gpsimd.indirect_dma_start(
                    out=x_sb[0:128, 400 * tb: 400 * tb + 300],
                    out_offset=None,
                    in_=wemb[:, :],
                    in_offset=IndirectOffsetOnAxis(
                        ap=idx_sb[0:128, tb:tb + 1], axis=0))
                nc.gpsimd.indirect_dma_start(
                    out=x_sb[0:128, 400 * tb + 300: 400 * tb + 400],
                    out_offset=None,
                    in_=pemb[:, :],
                    in_offset=IndirectOffsetOnAxis(
                        ap=idx_sb[0:128, 2 + tb:3 + tb], axis=0))
            nc.sync.dma_start(out=bias_sb[0][:, :], in_=bias0d[0])
            for d in range(2):
                nc.sync.dma_start(out=wih0_sb[d][:, :], in_=wih0d[d])
            nc.sync.dma_start(out=bias_sb[1][:, :], in_=bias1d[0])
            nc.sync.dma_start(out=idn100[:, :], in_=idnd[:, :])
            nc.sync.dma_start(out=uh_sb[:, :], in_=uhd[:, :])
            nc.sync.dma_start(out=um_sb[:, :], in_=umd[:, :])
            nc.sync.dma_start(out=w2_sb[:, :], in_=w2d[:, :])
            nc.sync.dma_start(out=b1_sb[:, :], in_=b1d[:, :])
            nc.sync.dma_start(out=b2_sb[:, :], in_=b2d[:, :])
            nc.sync.dma_start(out=selT_sb[0:128, 0:32], in_=selTd[0])
            nc.sync.dma_start(out=selT_sb[0:128, 32:64], in_=selTd[1])
            for dl in range(4):
                nc.sync.dma_start(out=whh_sb[dl][:, :], in_=whhd[dl])
            for d in range(2):
                nc.sync.dma_start(out=wih1_sb[d][:, :], in_=wih1d[d])
            for tb in range(2):
                for ch in range(4):
                    ptr = eps.tile([128, 128], F32, name="ptr", tag="ptr")
                    nc.tensor.transpose(
                        out=ptr[0:100, 0:128],
                        in_=x_sb[0:128, 400 * tb + 100 * ch: 400 * tb + 100 * ch + 100],
                        identity=idn128[:, :])
                    nc.vector.tensor_copy(
                        out=xT[0:100, 256 * ch + 128 * tb: 256 * ch + 128 * tb + 128],
                        in_=ptr[0:100, 0:128])

        # =========== xg for layer 0 ===========
        with tc.tile_pool(name="xg0ps", bufs=2, space="PSUM") as xps:
            for d in range(2):
                for half in range(2):
                    ps = xps.tile([128, 2048], F32, name="xg0ps", tag="xg0ps")
                    for tl in range(8):
                        tt = 8 * half + tl
                        for ch in range(4):
                            nc.tensor.matmul(
                                ps[0:100, 256 * tl: 256 * tl + 256],
                                lhsT=wih0_sb[d][0:100, 400 * tt + 100 * ch: 400 * tt + 100 * ch + 100],
                                rhs=xT[0:100, 256 * ch: 256 * ch + 256],
                                start=(ch == 0), stop=False,
                                skip_group_check=True)
                        nc.tensor.matmul(
                            ps[0:100, 256 * tl: 256 * tl + 256],
                            lhsT=bias_sb[0][0:1, 1600 * d + 100 * tt: 1600 * d + 100 * tt + 100],
                            rhs=ones_sb[0:1, 0:256],
                            start=False, stop=True, skip_group_check=True)
                    if half == 0:
                        nc.vector.tensor_copy(
                            out=xgT[0][d][0:100, 0:2048],
                            in_=ps[0:100, 0:2048])
                    else:
                        nc.scalar.copy(
                            out=xgT[0][d][0:100, 2048:4096],
                            in_=ps[0:100, 0:2048])
        w0ctx.__exit__(None, None, None)

        # =========== Jacobi sweep emitter ===========
        def emit_sweeps(l):
            with tc.tile_pool(name=f"sg{l}", bufs=1) as sgp, \
                 tc.tile_pool(name=f"scr{l}", bufs=1) as scr, \
                 tc.tile_pool(name=f"gps{l}", bufs=2, space="PSUM") as gps:
                for k in range(K_SWEEPS):
                    for d in range(2):
                        dl = 2 * l + d
                        sg = sgp.tile([100, 4096], F32, name=f"sg{d}", tag="sg")
                        if k == 0:
                            src = [xgT[l][d][0:100, 0:1024],
                                   xgT[l][d][0:100, 1024:2048],
                                   xgT[l][d][0:100, 2048:3072],
                                   xgT[l][d][0:100, 3072:4096]]
                        else:
                            src = []
                            for half in range(2):
                                ps = gps.tile([128, 2048], F32, name="gps", tag="gps")
                                for q in range(4):
                                    nc.tensor.matmul(
                                        ps[0:100, 512 * q: 512 * q + 512],
                                        lhsT=idn100[0:100, 0:100],
                                        rhs=xgT[l][d][0:100, 2048 * half + 512 * q: 2048 * half + 512 * q + 512],
                                        start=True, stop=False,
                                        skip_group_check=True)
                                for tl in range(8):
                                    tt = 8 * half + tl
                                    for j in range(4):
                                        # h_{t-1} (fwd) / h_{t+1} (bwd) via guard cols
                                        o0 = 0 if d == 0 else 2
                                        nc.tensor.matmul(
                                            ps[0:100, 256 * tl: 256 * tl + 256],
                                            lhsT=whh_sb[dl][0:100, 400 * tt + 100 * j: 400 * tt + 100 * j + 100],
                                            rhs=H[l][d][0:100, j, o0: o0 + 256],
                                            start=False, stop=(j == 3),
                                            skip_group_check=True)
                                src.append(ps[0:100, 0:1024])
                                src.append(ps[0:100, 1024:2048])
                        # i: sigmoid, g: tanh, f: sigmoid (before o), o: sigmoid
                        nc.scalar.activation(sg[0:100, 0:1024], src[0], AF.Sigmoid)
                        nc.scalar.activation(sg[0:100, 1024:2048], src[1], AF.Tanh)
                        nc.scalar.activation(sg[0:100, 2048:3072], src[2], AF.Sigmoid)
                        nc.scalar.activation(sg[0:100, 3072:4096], src[3], AF.Sigmoid)
                        u = scr.tile([100, 1024], F32, name=f"u{d}", tag=f"u{d}")
                        c = scr.tile([100, 1024], F32, name=f"c{d}", tag=f"c{d}")
                        thc = scr.tile([100, 1024], F32, name=f"th{d}", tag=f"th{d}")
                        nc.vector.tensor_tensor(
                            out=u[0:100, 0:1024], in0=sg[0:100, 0:1024],
                            in1=sg[0:100, 1024:2048], op=OP.mult)
                        for j in range(4):
                            if d == 0:
                                nc.vector.tensor_tensor_scan(
                                    out=c[0:100, 256 * j: 256 * j + 256],
                                    data0=sg[0:100, 2048 + 256 * j: 2304 + 256 * j],
                                    data1=u[0:100, 256 * j: 256 * j + 256],
                                    initial=0.0, op0=OP.mult, op1=OP.add)
                            else:
                                e1 = 256 * j - 1
                                nc.vector.tensor_tensor_scan(
                                    out=c[0:100, 256 * j + 255: (e1 if e1 >= 0 else None): -1],
                                    data0=sg[0:100, 2303 + 256 * j: 2047 + 256 * j: -1],
                                    data1=u[0:100, 256 * j + 255: (e1 if e1 >= 0 else None): -1],
                                    initial=0.0, op0=OP.mult, op1=OP.add)
                        nc.scalar.activation(thc[0:100, 0:1024], c[0:100, 0:1024], AF.Tanh)
                        nc.vector.tensor_tensor(
                            out=H[l][d][0:100, 0:4, 1:257],
                            in0=sg[0:100, 3072:4096], in1=thc[0:100, 0:1024],
                            op=OP.mult)

        emit_sweeps(0)

        # =========== xg for layer 1 (from H0) ===========
        with tc.tile_pool(name="xg1ps", bufs=2, space="PSUM") as xps:
            for d in range(2):
                for half in range(2):
                    ps = xps.tile([128, 2048], F32, name="xg1ps", tag="xg1ps")
                    for tl in range(8):
                        tt = 8 * half + tl
                        for ch in range(8):
                            dd, j = divmod(ch, 4)
                            nc.tensor.matmul(
                                ps[0:100, 256 * tl: 256 * tl + 256],
                                lhsT=wih1_sb[d][0:100, 800 * tt + 100 * ch: 800 * tt + 100 * ch + 100],
                                rhs=H[0][dd][0:100, j, 1:257],
                                start=(ch == 0), stop=False,
                                skip_group_check=True)
                        nc.tensor.matmul(
                            ps[0:100, 256 * tl: 256 * tl + 256],
                            lhsT=bias_sb[1][0:1, 1600 * d + 100 * tt: 1600 * d + 100 * tt + 100],
                            rhs=ones_sb[0:1, 0:256],
                            start=False, stop=True, skip_group_check=True)
                    if half == 0:
                        nc.vector.tensor_copy(
                            out=xgT[1][d][0:100, 0:2048], in_=ps[0:100, 0:2048])
                    else:
                        nc.scalar.copy(
                            out=xgT[1][d][0:100, 2048:4096], in_=ps[0:100, 0:2048])

        emit_sweeps(1)

        # =========== edge scorer ===========
        with tc.tile_pool(name="edge", bufs=1) as ep, \
             tc.tile_pool(name="edgeth", bufs=3) as thp, \
             tc.tile_pool(name="edgeps", bufs=1, space="PSUM") as epps, \
             tc.tile_pool(name="edgept", bufs=1, space="PSUM") as ptps:
            # B^T [100 f, 256 m] = Um^T @ h1cat (b1 folded into A side)
            pB = epps.tile([128, 256], F32, name="pB", tag="pB")
            for c in range(8):
                dd, j = divmod(c, 4)
                nc.tensor.matmul(
                    pB[0:100, 0:256],
                    lhsT=um_sb[0:100, 100 * c: 100 * c + 100],
                    rhs=H[1][dd][0:100, j, 1:257],
                    start=(c == 0), stop=(c == 7))
            # A^T [100 f, 256 t]
            pA = epps.tile([128, 256], F32, name="pA", tag="pA")
            for c in range(8):
                dd, j = divmod(c, 4)
                nc.tensor.matmul(
                    pA[0:100, 0:256],
                    lhsT=uh_sb[0:100, 100 * c: 100 * c + 100],
                    rhs=H[1][dd][0:100, j, 1:257],
                    start=(c == 0), stop=(c == 7))
            A_sb = ep.tile([100, 256], F32, name="A", tag="A")
            nc.vector.tensor_copy(out=A_sb[0:100, 0:256], in_=pA[0:100, 0:256])
            # select this core's 32 rows: transpose A^T chunks then selT matmul
            At_sb = ep.tile([128, 256], F32, name="At", tag="At")
            for m in range(2):
                pt = ptps.tile([128, 128], F32, name="pt", tag="pt")
                nc.tensor.transpose(
                    out=pt[0:128, 0:100],
                    in_=A_sb[0:100, 128 * m: 128 * m + 128],
                    identity=idn128[0:100, 0:100])
                nc.vector.tensor_copy(
                    out=At_sb[0:128, 128 * m: 128 * m + 100],
                    in_=pt[0:128, 0:100])
            pS = ptps.tile([128, 32], F32, name="pS", tag="pS")
            for m in range(2):
                nc.tensor.matmul(
                    pS[0:100, 0:32],
                    lhsT=At_sb[0:128, 128 * m: 128 * m + 100],
                    rhs=selT_sb[0:128, 32 * m: 32 * m + 32],
                    start=(m == 0), stop=(m == 1))
            ATb = ep.tile([100, 32], F32, name="ATb", tag="ATb")
            nc.vector.tensor_scalar(
                out=ATb[0:100, 0:32], in0=pS[0:100, 0:32],
                scalar1=b1_sb[0:100, 0:1], scalar2=None, op0=OP.add)

            psS_tiles = [epps.tile([128, 512], F32, name=f"psS{q}", tag=f"psS{q}")
                         for q in range(4)]
            for q in range(4):
                nc.vector.memset(psS_tiles[q][:, :], 0.0)
            gsb_tiles = [ep.tile([128, 512], F32, name=f"gsb{q}", tag=f"gsb{q}")
                         for q in range(4)]
            for r in range(32):
                th_t = thp.tile([100, 256], BF16, name="th", tag="th")
                nc.scalar.activation(
                    th_t[0:100, 0:256], pB[0:100, 0:256], AF.Tanh,
                    bias=ATb[0:100, r:r + 1], scale=1.0)
                q, half = divmod(r // 4, 2)
                nc.tensor.matmul(
                    psS_tiles[q][32 * (r % 4): 32 * (r % 4) + 1,
                                 256 * half: 256 * half + 256],
                    lhsT=w2_sb[0:100, 0:1],
                    rhs=th_t[0:100, 0:256],
                    start=True, stop=True,
                    skip_group_check=True,
                    tile_position=(0, 32 * (r % 4)))
                if r % 8 == 7:
                    # quadrant q complete -> write back while later rows run
                    nc.vector.tensor_scalar(
                        out=gsb_tiles[q][0:128, 0:512],
                        in0=psS_tiles[q][0:128, 0:512],
                        scalar1=b2_sb[0:128, 0:1], scalar2=None, op0=OP.add)
                    for hh in range(2):
                        rb = 4 * (2 * q + hh)
                        nc.sync.dma_start(
                            out=grid[rb:rb + 4, 0:256],
                            in_=gsb_tiles[q][0:128:32, 256 * hh: 256 * hh + 256])

    nc.compile()
    return nc


_NC_CACHE = None


def _get_nc():
    global _NC_CACHE
    if _NC_CACHE is None:
        _NC_CACHE = build_nc()
    return _NC_CACHE


def kernel(**inputs) -> np.ndarray:
    from concourse.bass_utils import run_bass_kernel_spmd

    arr = _prep_inputs(**inputs)
    nc = _get_nc()
    in_maps = []
    for k in range(NC):
        m = dict(arr)
        m["selT"] = _make_selT(k)
        in_maps.append(m)
    res = run_bass_kernel_spmd(nc, in_maps, core_ids=list(range(NC)))
    grid = np.concatenate([res.results[k]["grid"] for k in range(NC)], axis=0)
    mask = np.ones((N, N), dtype=bool)
    np.fill_diagonal(mask, False)
    mask[:, 0] = False
    return grid[mask].reshape(-1, 1).astype(np.float32)
